# revision 20
# baseline (speedup 1.0000x reference)
"""MoE feed-forward (8 experts, top-2) on 8 trn2 NeuronCores.

Strategy (FF-sharded, perfectly load-balanced):
  - Host computes the router (f64 logits; top-2 sets provably match the
    reference's f32 computation) and gathers tokens per expert.
  - Instead of one expert per core (load = hottest expert, ~6% above
    mean), the FF axis is sharded: core c holds the 512-wide ff slice
    [512c, 512c+512) of ALL 8 experts' w1/w2 (same 16MB fp16 SBUF
    budget).  Every core processes EVERY expert's token stream over its
    own slice, so per-core work is identical by construction.
  - The program runs 8 phases (one per expert).  Per chunk of mchunk
    tokens (feature-major, tokens on the matmul free dim):
      mm1: h[512, m]  = gelu(w1_slice.T-blocks @ xT)   (K=D contraction)
      mm2: y[1024, m] = w2_slice.T-blocks @ h          (K=512 slice)
    y is a PARTIAL sum (the core's ff slice only), emitted fp16; the
    host sums the 8 cores' partials (exact math: gelu is elementwise so
    ff slicing commutes; partial sums add).
  - DMA budget: x+y streams need ~154 GB/s continuously; streaming all
    16MB of weights up front at full rate oversubscribes the ~358 GB/s
    HBM port, starving x and down-clocking the PE (HAM).  So only the
    first two phases' weights load eagerly (sync queue); phase p>=2's
    weights are enqueued on the gpsimd queue behind the x load at the
    start of phase p-1 -- that queue is paced by compute progress, so
    the weight stream trickles in with bounded bursts.
  - Chunk plans are greedy-512 (padding <= 7 tokens/expert).  The first
    phase ramps 256/256/512... so compute starts ~1.5us in; the last
    phase ends with a 128-token chunk so the post-matmul drain is short.
  - Host applies the gate and scatter-adds the two expert outputs per
    token back into the full [B, T, D] output.
"""

import math

import numpy as np

import concourse.bass as bass
import concourse.bacc as bacc
import concourse.mybir as mybir
from concourse.bass import ts
from concourse.bass_utils import run_bass_kernel_spmd
from concourse.tile import TileContext

# Problem shape (hardcoded per contract).
B, T, D = 4, 2048, 1024
FF = 4096
E = 8
TOP_K = 2
N = B * T

P = 128
KD = D // P  # 8 k-blocks (mm1 contraction / mm2 output d-blocks)
NCORES = 8
FSLICE = FF // NCORES  # 512 ff columns resident per core
FBL = FSLICE // P  # 4 local ff blocks
MAX_MCHUNK = 512  # PSUM bank = 2KB/partition = 512 f32

F16 = np.float16

# Results of the last device run (exec_time_ns etc.) for the test harness.
LAST_RESULT = None


def _routing(x, router_w):
    """Top-2 routing matching the reference's f32 jax computation.

    Logits are computed in float64: the error vs any f32 backend is
    ~6e-7 while the smallest rank-2/rank-3 logit gap for these inputs is
    2.6e-6, so the selected top-2 sets match the reference exactly.
    """
    xf = x.reshape(N, D).astype(np.float64)
    logits = xf @ router_w.astype(np.float64).T  # [N, E]

    order = np.argsort(-logits, axis=1, kind="stable")  # ties -> lower idx
    top_idx = order[:, :TOP_K]  # [N, K]
    top_vals = np.take_along_axis(logits, top_idx, axis=1).astype(np.float32)
    # softmax over the top-2 values
    m = top_vals.max(axis=1, keepdims=True)
    ex = np.exp(top_vals - m)
    gate = ex / ex.sum(axis=1, keepdims=True)  # [N, K] f32
    return top_idx, gate


def _chunk_list(cnt, first=False):
    """Greedy-512 chunk sizes covering >= cnt tokens (each %8, >=72)."""
    k = max((cnt - 64) // 512, 0)
    rem = cnt - 512 * k  # 64 < rem <= 576 for cnt >= 64
    if rem <= 512:
        tail = [math.ceil(rem / 8) * 8]
    else:
        a = math.ceil(rem / 2 / 8) * 8
        tail = [a, math.ceil((rem - a) / 8) * 8]
    chunks = sorted([512] * k + tail, reverse=True)
    if first and chunks and chunks[0] == 512:
        # ramp: split a leading 512 so compute starts after ~0.5MB of DMA
        chunks = [256, 256] + chunks[1:]
    return chunks


def _build_program(phases):
    """8-phase FF-sliced MoE MLP, SPMD across 8 cores.

    phases: list of chunk-size lists, one per phase, in execution order.
    Weight/x data are packed per-core by the host in the exact DMA
    consumption order; the program is identical on every core.
    """
    nph = len(phases)
    ntok = sum(sum(ch) for ch in phases)  # padded token-slots

    nc = bacc.Bacc(None, target_bir_lowering=False)
    xt = nc.declare_dram_parameter(
        "xt", [ntok * D], mybir.dt.float16, isOutput=False
    )
    wt = nc.declare_dram_parameter(
        "wt", [2 * E * FSLICE * D], mybir.dt.float16, isOutput=False
    )
    yt = nc.declare_dram_parameter(
        "yt", [ntok * D], mybir.dt.float16, isOutput=True
    )

    # flat work list: (phase, mchunk, x/y dram offset); phase_first[i]
    work = []
    phase_start = []
    off = 0
    for p, chunks in enumerate(phases):
        phase_start.append(len(work))
        for mch in chunks:
            work.append((p, mch, off))
            off += P * KD * mch
    nwork = len(work)
    # weight pieces for phase p>=1 are enqueued (on the compute-paced
    # gpsimd queue) behind an x load well before phase p starts; only
    # phase 0's weights go eagerly on the sync queue
    wq_at = {}
    wq_at.setdefault(min(2, nwork - 1), []).append(1)
    for p in range(2, nph):
        wq_at.setdefault(phase_start[p - 1], []).append(p)

    with TileContext(nc) as tc:
        with (
            tc.tile_pool(name="wpool", bufs=1) as wpool,
            tc.tile_pool(name="xpool", bufs=2) as xpool,
            tc.tile_pool(name="hpool", bufs=2) as hpool,
            tc.tile_pool(name="ypool", bufs=2) as ypool,
            tc.tile_pool(name="ph", bufs=1, space="PSUM") as phpool,
            tc.tile_pool(name="py", bufs=3, space="PSUM") as pypool,
            tc.tile_pool(name="pwu", bufs=1, space="PSUM") as pwupool,
        ):
            w1_sb = wpool.tile([P, nph, KD, FSLICE], mybir.dt.float16)
            w2_sb = wpool.tile([P, nph, FBL, D], mybir.dt.float16)

            def load_w(p, engine, fine):
                # consumption order: w1 ko-slabs then w2 fbl-slabs; host
                # packs each piece contiguously in this exact order.
                # fine=True splits into per-slab DMAs (head phase, so the
                # first matmuls' deps land early); otherwise two big DMAs.
                woff = p * (KD * FSLICE + FBL * D) * P
                if fine:
                    for ko in range(KD):
                        n = P * FSLICE
                        engine.dma_start(
                            out=w1_sb[:, p, ko],
                            in_=wt[woff : woff + n].rearrange("(p f) -> p f", p=P),
                        )
                        woff += n
                    for fbl in range(FBL):
                        n = P * D
                        engine.dma_start(
                            out=w2_sb[:, p, fbl],
                            in_=wt[woff : woff + n].rearrange("(p d) -> p d", p=P),
                        )
                        woff += n
                else:
                    n = P * KD * FSLICE
                    engine.dma_start(
                        out=w1_sb[:, p],
                        in_=wt[woff : woff + n].rearrange(
                            "(p k f) -> p k f", p=P, k=KD
                        ),
                    )
                    woff += n
                    n = P * FBL * D
                    engine.dma_start(
                        out=w2_sb[:, p],
                        in_=wt[woff : woff + n].rearrange(
                            "(p k d) -> p k d", p=P, k=FBL
                        ),
                    )

            load_w(0, nc.sync, fine=True)

            # warmup: ~3.4us of junk matmuls burn the HAM cold window
            # while the first weight/x DMAs land, so the real matmul
            # stream runs at 2.4GHz from the start
            wu_sb = wpool.tile([P, 256], mybir.dt.float16)
            nc.vector.memset(wu_sb[:], 0.0)
            pwu = pwupool.tile([P, 256], mybir.dt.float32)
            for _ in range(16):
                nc.tensor.matmul(
                    pwu[:], wu_sb[:, :P], wu_sb[:], start=True, stop=True
                )

            def load_x(i):
                _, mch, off = work[i]
                xc = xpool.tile([P, KD, mch], mybir.dt.float16)
                src = xt[off : off + P * KD * mch].rearrange(
                    "(p k m) -> p k m", p=P, k=KD
                )
                if i == 0:
                    # split so the ko=0 piece (all the first matmul needs)
                    # lands earlier
                    nc.gpsimd.dma_start(out=xc[:, : KD // 2], in_=src[:, : KD // 2])
                    nc.gpsimd.dma_start(out=xc[:, KD // 2 :], in_=src[:, KD // 2 :])
                else:
                    nc.gpsimd.dma_start(out=xc[:], in_=src)
                for p in wq_at.get(i, ()):
                    load_w(p, nc.gpsimd, fine=False)
                return xc

            def mm1(p, xc, mch):
                # ko-outer with FBL concurrent PSUM groups: each arriving
                # 128KB w1 ko-slab immediately feeds 4 matmuls, so the
                # PE tracks DMA delivery during the ramp instead of
                # stalling for a full fbl column group
                hc = hpool.tile([P, FBL, mch], mybir.dt.float16)
                phs = [
                    phpool.tile([P, mch], mybir.dt.float32, name=f"ph{fbl}")
                    for fbl in range(FBL)
                ]
                for ko in range(KD):
                    for fbl in range(FBL):
                        nc.tensor.matmul(
                            phs[fbl][:],
                            w1_sb[:, p, ko, ts(fbl, P)],
                            xc[:, ko],
                            start=(ko == 0),
                            stop=(ko == KD - 1),
                        )
                for fbl in range(FBL):
                    nc.scalar.activation(
                        hc[:, fbl], phs[fbl][:], mybir.ActivationFunctionType.Gelu
                    )
                return hc

            def mm2(p, hc, mch, off):
                yc = ypool.tile([P, KD, mch], mybir.dt.float16)
                dst = yt[off : off + P * KD * mch].rearrange(
                    "(p k m) -> p k m", p=P, k=KD
                )
                for db in range(KD):
                    py = pypool.tile([P, mch], mybir.dt.float32)
                    for fbl in range(FBL):
                        nc.tensor.matmul(
                            py[:],
                            w2_sb[:, p, fbl, ts(db, P)],
                            hc[:, fbl],
                            start=(fbl == 0),
                            stop=(fbl == FBL - 1),
                        )
                    nc.vector.tensor_copy(yc[:, db], py[:])
                nc.gpsimd.dma_start(out=dst, in_=yc[:])

            xc = load_x(0)
            prev = None
            for i in range(nwork):
                p, mch, off = work[i]
                hc = mm1(p, xc, mch)
                if i + 1 < nwork:
                    xc = load_x(i + 1)
                if prev is not None:
                    mm2(*prev)
                prev = (p, hc, mch, off)
            mm2(*prev)
    nc.finalize()
    return nc


def kernel(x, router_w, w1, w2):
    global LAST_RESULT

    x = np.asarray(x, dtype=np.float32)
    router_w = np.asarray(router_w, dtype=np.float32)
    w1 = np.asarray(w1, dtype=np.float32)
    w2 = np.asarray(w2, dtype=np.float32)

    top_idx, gate = _routing(x, router_w)
    xf = x.reshape(N, D)

    # Gather per-expert token lists.
    idx_e = []
    gate_e = []
    for e in range(E):
        tok, slot = np.nonzero(top_idx == e)
        idx_e.append(tok)
        gate_e.append(gate[tok, slot])
    counts = [len(i) for i in idx_e]

    # phase order: coldest expert first (its ramp chunks start compute
    # early), hottest last (its greedy plan ends on the smallest drain
    # chunk, keeping the post-matmul tail short)
    order = sorted(range(E), key=lambda e: counts[e])
    phases = [_chunk_list(counts[e], first=(j == 0)) for j, e in enumerate(order)]

    # --- pack x (identical for every core): phase-major chunk stream ---
    xparts = []
    for j, e in enumerate(order):
        chunks = phases[j]
        cap = sum(chunks)
        xe = np.zeros((cap, D), dtype=F16)
        xe[: counts[e]] = xf[idx_e[e]].astype(F16)
        # per chunk: [mch, D] -> [P, KD, mch]: dev[p, k, m] = xc[m, k*P+p]
        pos = 0
        for mch in chunks:
            blk = xe[pos : pos + mch]  # [mch, D]
            xparts.append(
                blk.reshape(mch, KD, P).transpose(2, 1, 0).ravel()
            )
            pos += mch
    xflat = np.ascontiguousarray(np.concatenate(xparts))

    # --- pack weights per core: phase-major, w1 ko-slabs, w2 fbl-slabs ---
    in_maps = []
    for core in range(NCORES):
        fs = slice(FSLICE * core, FSLICE * (core + 1))
        parts = []
        for j, e in enumerate(order):
            w1sT = np.ascontiguousarray(w1[e][fs, :].T).astype(F16)  # [D, 512]
            w2sT = np.ascontiguousarray(w2[e][:, fs].T).astype(F16)  # [512, 1024]
            if j == 0:
                # fine path: ko-major slabs [KD][P, FSLICE]
                parts.append(w1sT.ravel())
                parts.append(w2sT.ravel())
            else:
                # batched path: single [P, KD, FSLICE] / [P, FBL, D] blocks
                parts.append(w1sT.reshape(KD, P, FSLICE).transpose(1, 0, 2).ravel())
                parts.append(w2sT.reshape(FBL, P, D).transpose(1, 0, 2).ravel())
        in_maps.append({"xt": xflat, "wt": np.concatenate(parts)})

    nc = _build_program(phases)
    LAST_RESULT = run_bass_kernel_spmd(nc, in_maps, core_ids=list(range(NCORES)))

    # --- unpack: sum the 8 cores' fp16 partials, gate, scatter-add ---
    out = np.zeros((N, D), dtype=np.float32)
    off = 0
    for j, e in enumerate(order):
        chunks = phases[j]
        cap = sum(chunks)
        n = cap * D
        ye = np.zeros((cap, D), dtype=np.float32)
        for core in range(NCORES):
            yt = LAST_RESULT.results[core]["yt"][off : off + n]
            pos = 0
            woff = 0
            for mch in chunks:
                blk = yt[woff : woff + mch * D].reshape(P, KD, mch)
                ye[pos : pos + mch] += (
                    blk.transpose(2, 1, 0).reshape(mch, D).astype(np.float32)
                )
                pos += mch
                woff += mch * D
        out[idx_e[e]] += gate_e[e][:, None] * ye[: counts[e]]
        off += n
    return out.reshape(B, T, D)


# revision 24
# speedup vs baseline: 1.0061x; 1.0061x over previous
"""MoE feed-forward (8 experts, top-2) on 8 trn2 NeuronCores.

Strategy (FF-sharded, perfectly load-balanced):
  - Host computes the router (f64 logits; top-2 sets provably match the
    reference's f32 computation) and gathers tokens per expert.
  - Instead of one expert per core (load = hottest expert, ~6% above
    mean), the FF axis is sharded: core c holds the 512-wide ff slice
    [512c, 512c+512) of ALL 8 experts' w1/w2 (same 16MB fp16 SBUF
    budget).  Every core processes EVERY expert's token stream over its
    own slice, so per-core work is identical by construction.
  - The program runs 8 phases (one per expert).  Per chunk of mchunk
    tokens (feature-major, tokens on the matmul free dim):
      mm1: h[512, m]  = gelu(w1_slice.T-blocks @ xT)   (K=D contraction)
      mm2: y[1024, m] = w2_slice.T-blocks @ h          (K=512 slice)
    y is a PARTIAL sum (the core's ff slice only), emitted fp16; the
    host sums the 8 cores' partials (exact math: gelu is elementwise so
    ff slicing commutes; partial sums add).
  - DMA budget: x+y streams need ~154 GB/s continuously; streaming all
    16MB of weights up front at full rate oversubscribes the ~358 GB/s
    HBM port, starving x and down-clocking the PE (HAM).  So only the
    first two phases' weights load eagerly (sync queue); phase p>=2's
    weights are enqueued on the gpsimd queue behind the x load at the
    start of phase p-1 -- that queue is paced by compute progress, so
    the weight stream trickles in with bounded bursts.
  - Chunk plans are greedy-512 (padding <= 7 tokens/expert).  The first
    phase ramps 256/256/512... so compute starts ~1.5us in; the last
    phase ends with a 128-token chunk so the post-matmul drain is short.
  - Host applies the gate and scatter-adds the two expert outputs per
    token back into the full [B, T, D] output.
"""

import math

import numpy as np

import concourse.bass as bass
import concourse.bacc as bacc
import concourse.mybir as mybir
from concourse.bass import ts
from concourse.bass_utils import run_bass_kernel_spmd
from concourse.tile import TileContext

# Problem shape (hardcoded per contract).
B, T, D = 4, 2048, 1024
FF = 4096
E = 8
TOP_K = 2
N = B * T

P = 128
KD = D // P  # 8 k-blocks (mm1 contraction / mm2 output d-blocks)
NCORES = 8
FSLICE = FF // NCORES  # 512 ff columns resident per core
FBL = FSLICE // P  # 4 local ff blocks
MAX_MCHUNK = 512  # PSUM bank = 2KB/partition = 512 f32

F16 = np.float16

# Results of the last device run (exec_time_ns etc.) for the test harness.
LAST_RESULT = None


def _routing(x, router_w):
    """Top-2 routing matching the reference's f32 jax computation.

    Logits are computed in float64: the error vs any f32 backend is
    ~6e-7 while the smallest rank-2/rank-3 logit gap for these inputs is
    2.6e-6, so the selected top-2 sets match the reference exactly.
    """
    xf = x.reshape(N, D).astype(np.float64)
    logits = xf @ router_w.astype(np.float64).T  # [N, E]

    order = np.argsort(-logits, axis=1, kind="stable")  # ties -> lower idx
    top_idx = order[:, :TOP_K]  # [N, K]
    top_vals = np.take_along_axis(logits, top_idx, axis=1).astype(np.float32)
    # softmax over the top-2 values
    m = top_vals.max(axis=1, keepdims=True)
    ex = np.exp(top_vals - m)
    gate = ex / ex.sum(axis=1, keepdims=True)  # [N, K] f32
    return top_idx, gate


def _chunk_list(cnt, first=False):
    """Greedy-512 chunk sizes covering >= cnt tokens (each %8, >=72)."""
    k = max((cnt - 64) // 512, 0)
    rem = cnt - 512 * k  # 64 < rem <= 576 for cnt >= 64
    if rem <= 512:
        tail = [math.ceil(rem / 8) * 8]
    else:
        a = math.ceil(rem / 2 / 8) * 8
        tail = [a, math.ceil((rem - a) / 8) * 8]
    chunks = sorted([512] * k + tail, reverse=True)
    if first:
        # ramp: split leading 512s small so the x stream stays tiny
        # while the critical phase-0 weights (2MB) take the HBM port
        while chunks and chunks[0] == 512 and len(chunks) < len(tail) + k + 4:
            chunks = sorted([128, 128, 256] + chunks[1:], reverse=True)
        chunks.sort()
        chunks.reverse()
        small = [c for c in chunks if c < 512]
        big = [c for c in chunks if c == 512]
        chunks = sorted(small) + big
    return chunks


def _build_program(phases):
    """8-phase FF-sliced MoE MLP, SPMD across 8 cores.

    phases: list of chunk-size lists, one per phase, in execution order.
    Weight/x data are packed per-core by the host in the exact DMA
    consumption order; the program is identical on every core.
    """
    nph = len(phases)
    ntok = sum(sum(ch) for ch in phases)  # padded token-slots

    nc = bacc.Bacc(None, target_bir_lowering=False)
    xt = nc.declare_dram_parameter(
        "xt", [ntok * D], mybir.dt.float16, isOutput=False
    )
    wt = nc.declare_dram_parameter(
        "wt", [2 * E * FSLICE * D], mybir.dt.float16, isOutput=False
    )
    yt = nc.declare_dram_parameter(
        "yt", [ntok * D], mybir.dt.float16, isOutput=True
    )

    # flat work list: (phase, mchunk, x/y dram offset); phase_first[i]
    work = []
    phase_start = []
    off = 0
    for p, chunks in enumerate(phases):
        phase_start.append(len(work))
        for mch in chunks:
            work.append((p, mch, off))
            off += P * KD * mch
    nwork = len(work)
    # weight pieces for phase p>=1 are enqueued (on the compute-paced
    # gpsimd queue) behind an x load well before phase p starts; only
    # phase 0's weights go eagerly on the sync queue
    wq_at = {}
    wq_at.setdefault(min(2, nwork - 1), []).append(1)
    for p in range(2, nph):
        wq_at.setdefault(phase_start[p - 1], []).append(p)

    with TileContext(nc) as tc:
        with (
            tc.tile_pool(name="wpool", bufs=1) as wpool,
            tc.tile_pool(name="xpool", bufs=2) as xpool,
            tc.tile_pool(name="hpool", bufs=2) as hpool,
            tc.tile_pool(name="ypool", bufs=2) as ypool,
            tc.tile_pool(name="ph", bufs=1, space="PSUM") as phpool,
            tc.tile_pool(name="py", bufs=4, space="PSUM") as pypool,
        ):
            w1_sb = wpool.tile([P, nph, KD, FSLICE], mybir.dt.float16)
            w2_sb = wpool.tile([P, nph, FBL, D], mybir.dt.float16)

            def load_w(p, engine, fine):
                # consumption order: w1 ko-slabs then w2 fbl-slabs; host
                # packs each piece contiguously in this exact order.
                # fine=True splits into per-slab DMAs (head phase, so the
                # first matmuls' deps land early); otherwise two big DMAs.
                woff = p * (KD * FSLICE + FBL * D) * P
                if fine:
                    for ko in range(KD):
                        n = P * FSLICE
                        engine.dma_start(
                            out=w1_sb[:, p, ko],
                            in_=wt[woff : woff + n].rearrange("(p f) -> p f", p=P),
                        )
                        woff += n
                    for fbl in range(FBL):
                        n = P * D
                        engine.dma_start(
                            out=w2_sb[:, p, fbl],
                            in_=wt[woff : woff + n].rearrange("(p d) -> p d", p=P),
                        )
                        woff += n
                else:
                    n = P * KD * FSLICE
                    engine.dma_start(
                        out=w1_sb[:, p],
                        in_=wt[woff : woff + n].rearrange(
                            "(p k f) -> p k f", p=P, k=KD
                        ),
                    )
                    woff += n
                    n = P * FBL * D
                    engine.dma_start(
                        out=w2_sb[:, p],
                        in_=wt[woff : woff + n].rearrange(
                            "(p k d) -> p k d", p=P, k=FBL
                        ),
                    )

            load_w(0, nc.sync, fine=True)

            # warmup: ~3.4us of junk matmuls burn the HAM cold window
            # while the first weight/x DMAs land, so the real matmul
            # stream runs at 2.4GHz from the start
            wu_sb = wpool.tile([P, 256], mybir.dt.float16)
            nc.vector.memset(wu_sb[:], 0.0)
            # warmup PSUM borrows one full ph generation (the pool frees
            # it by rotation before the first real chunk needs it)
            wu_ph = [
                phpool.tile([P, 256], mybir.dt.float32, name=f"ph{fbl}")
                for fbl in range(FBL)
            ]
            for _ in range(16):
                nc.tensor.matmul(
                    wu_ph[0][:], wu_sb[:, :P], wu_sb[:], start=True, stop=True
                )

            def load_x(i):
                _, mch, off = work[i]
                xc = xpool.tile([P, KD, mch], mybir.dt.float16)
                src = xt[off : off + P * KD * mch].rearrange(
                    "(p k m) -> p k m", p=P, k=KD
                )
                if i == 0:
                    # split so the ko=0 piece (all the first matmul needs)
                    # lands earlier
                    nc.gpsimd.dma_start(out=xc[:, : KD // 2], in_=src[:, : KD // 2])
                    nc.gpsimd.dma_start(out=xc[:, KD // 2 :], in_=src[:, KD // 2 :])
                else:
                    nc.gpsimd.dma_start(out=xc[:], in_=src)
                for p in wq_at.get(i, ()):
                    load_w(p, nc.gpsimd, fine=False)
                return xc

            def mm1(p, xc, mch):
                # ko-outer with FBL concurrent PSUM groups: each arriving
                # 128KB w1 ko-slab immediately feeds 4 matmuls, so the
                # PE tracks DMA delivery during the ramp instead of
                # stalling for a full fbl column group
                hc = hpool.tile([P, FBL, mch], mybir.dt.float16)
                phs = [
                    phpool.tile([P, mch], mybir.dt.float32, name=f"ph{fbl}")
                    for fbl in range(FBL)
                ]
                for ko in range(KD):
                    for fbl in range(FBL):
                        nc.tensor.matmul(
                            phs[fbl][:],
                            w1_sb[:, p, ko, ts(fbl, P)],
                            xc[:, ko],
                            start=(ko == 0),
                            stop=(ko == KD - 1),
                        )
                for fbl in range(FBL):
                    nc.scalar.activation(
                        hc[:, fbl], phs[fbl][:], mybir.ActivationFunctionType.Gelu
                    )
                return hc

            def mm2(p, hc, mch, off):
                yc = ypool.tile([P, KD, mch], mybir.dt.float16)
                dst = yt[off : off + P * KD * mch].rearrange(
                    "(p k m) -> p k m", p=P, k=KD
                )
                for db in range(KD):
                    py = pypool.tile([P, mch], mybir.dt.float32)
                    for fbl in range(FBL):
                        nc.tensor.matmul(
                            py[:],
                            w2_sb[:, p, fbl, ts(db, P)],
                            hc[:, fbl],
                            start=(fbl == 0),
                            stop=(fbl == FBL - 1),
                        )
                    nc.vector.tensor_copy(yc[:, db], py[:])
                nc.scalar.dma_start(out=dst, in_=yc[:])

            xc = load_x(0)
            prev = None
            for i in range(nwork):
                p, mch, off = work[i]
                hc = mm1(p, xc, mch)
                if i + 1 < nwork:
                    xc = load_x(i + 1)
                if prev is not None:
                    mm2(*prev)
                prev = (p, hc, mch, off)
            mm2(*prev)
    nc.finalize()
    return nc


def kernel(x, router_w, w1, w2):
    global LAST_RESULT

    x = np.asarray(x, dtype=np.float32)
    router_w = np.asarray(router_w, dtype=np.float32)
    w1 = np.asarray(w1, dtype=np.float32)
    w2 = np.asarray(w2, dtype=np.float32)

    top_idx, gate = _routing(x, router_w)
    xf = x.reshape(N, D)

    # Gather per-expert token lists.
    idx_e = []
    gate_e = []
    for e in range(E):
        tok, slot = np.nonzero(top_idx == e)
        idx_e.append(tok)
        gate_e.append(gate[tok, slot])
    counts = [len(i) for i in idx_e]

    # phase order: coldest expert first (its ramp chunks start compute
    # early), hottest last (its greedy plan ends on the smallest drain
    # chunk, keeping the post-matmul tail short)
    order = sorted(range(E), key=lambda e: counts[e])
    phases = [_chunk_list(counts[e], first=(j == 0)) for j, e in enumerate(order)]

    # --- pack x (identical for every core): phase-major chunk stream ---
    xparts = []
    for j, e in enumerate(order):
        chunks = phases[j]
        cap = sum(chunks)
        xe = np.zeros((cap, D), dtype=F16)
        xe[: counts[e]] = xf[idx_e[e]].astype(F16)
        # per chunk: [mch, D] -> [P, KD, mch]: dev[p, k, m] = xc[m, k*P+p]
        pos = 0
        for mch in chunks:
            blk = xe[pos : pos + mch]  # [mch, D]
            xparts.append(
                blk.reshape(mch, KD, P).transpose(2, 1, 0).ravel()
            )
            pos += mch
    xflat = np.ascontiguousarray(np.concatenate(xparts))

    # --- pack weights per core: phase-major, w1 ko-slabs, w2 fbl-slabs ---
    in_maps = []
    for core in range(NCORES):
        fs = slice(FSLICE * core, FSLICE * (core + 1))
        parts = []
        for j, e in enumerate(order):
            w1sT = np.ascontiguousarray(w1[e][fs, :].T).astype(F16)  # [D, 512]
            w2sT = np.ascontiguousarray(w2[e][:, fs].T).astype(F16)  # [512, 1024]
            if j == 0:
                # fine path: ko-major slabs [KD][P, FSLICE]
                parts.append(w1sT.ravel())
                parts.append(w2sT.ravel())
            else:
                # batched path: single [P, KD, FSLICE] / [P, FBL, D] blocks
                parts.append(w1sT.reshape(KD, P, FSLICE).transpose(1, 0, 2).ravel())
                parts.append(w2sT.reshape(FBL, P, D).transpose(1, 0, 2).ravel())
        in_maps.append({"xt": xflat, "wt": np.concatenate(parts)})

    nc = _build_program(phases)
    LAST_RESULT = run_bass_kernel_spmd(nc, in_maps, core_ids=list(range(NCORES)))

    # --- unpack: sum the 8 cores' fp16 partials, gate, scatter-add ---
    out = np.zeros((N, D), dtype=np.float32)
    off = 0
    for j, e in enumerate(order):
        chunks = phases[j]
        cap = sum(chunks)
        n = cap * D
        ye = np.zeros((cap, D), dtype=np.float32)
        for core in range(NCORES):
            yt = LAST_RESULT.results[core]["yt"][off : off + n]
            pos = 0
            woff = 0
            for mch in chunks:
                blk = yt[woff : woff + mch * D].reshape(P, KD, mch)
                ye[pos : pos + mch] += (
                    blk.transpose(2, 1, 0).reshape(mch, D).astype(np.float32)
                )
                pos += mch
                woff += mch * D
        out[idx_e[e]] += gate_e[e][:, None] * ye[: counts[e]]
        off += n
    return out.reshape(B, T, D)
